# revision 15
# baseline (speedup 1.0000x reference)
"""Trainium2 Bass kernel for a 2-layer GCN encoder (GCNConv -> LN -> GELU -> GCNConv -> LN).

Strategy (8 NeuronCores, SPMD), v2:
  - Nodes assigned to 784 global tiles of 128 (degree-balanced); core k owns
    tiles t with t%8==k (98 tiles = 12544 dst rows per core).
  - Layer 1: every core computes the FULL transformed table H1*dinv locally
    (X@W1 on all 784 tiles; no collective). Per-core table row order is a
    per-core permutation with the core's own tiles LAST so "my rows" sit at a
    core-independent offset.
  - Aggregation: normalization folded into the table (rows pre-scaled by
    dinv[src]) and the output (post-scaled by dinv[dst]); selector matrices
    are pure one-hot (single is_equal). Self-loops handled by adding the
    node's own table row (sequential read), not as gather edges.
  - Gathers: bf16 rows via dma_gather in fixed 3072-idx multi-packet calls,
    fully padded (pad idx=0, pad slot=200 -> zero selector column), so no
    per-call count registers and no NaN-guard memsets.
  - Layer 2: transform locally (h1g @ W2, scaled by dinv), AllGather the
    bf16 table (core-major rows), aggregate the same way.
"""

from contextlib import ExitStack

import numpy as np

import concourse.bass as bass
import concourse.bacc as bacc
import concourse.mybir as mybir
import concourse.tile as tile
from concourse.bass_utils import run_bass_kernel_spmd

dt = mybir.dt
F32 = dt.float32
BF16 = dt.bfloat16

# -------- problem geometry (hardcoded for the graded problem) --------
N_FULL = 100000
IN_DIM = 256
HID2 = 256
HID = 128
N_CORES = 8
TILE = 128
NT = 784           # global tiles
TPC = 98           # tiles per core
SHARD = TPC * TILE # 12544
PADN = NT * TILE   # 100352
NCHUNK = 4
CH = PADN // NCHUNK  # 25088 (int16-safe)
CALLB = 24         # blocks per gather call (3072 idxs; multi-packet)
NI = CALLB * TILE  # 3072
MYBASE = (NT - TPC) * TILE  # 87808: per-core table rows of own tiles
PADSLOT = 200.0


# ============================ bass program builder ============================

def build_program(tc, io, geom):
    nc = tc.nc
    AOT = mybir.AluOpType
    AFT = mybir.ActivationFunctionType
    eps = 1e-5
    n_in_ch = IN_DIM // 128
    n_h_ch = HID2 // 128

    ctx = ExitStack()
    consts = ctx.enter_context(tc.tile_pool(name="consts", bufs=1))
    work = ctx.enter_context(tc.tile_pool(name="work", bufs=2))
    ln = ctx.enter_context(tc.tile_pool(name="ln", bufs=3))
    msgp = ctx.enter_context(tc.tile_pool(name="msgp", bufs=8))
    idxp = ctx.enter_context(tc.tile_pool(name="idxp", bufs=4))
    selp = ctx.enter_context(tc.tile_pool(name="selp", bufs=3))
    trowp = ctx.enter_context(tc.tile_pool(name="trowp", bufs=2))
    ps256 = ctx.enter_context(tc.tile_pool(name="ps256", bufs=2, space="PSUM"))
    ps128 = ctx.enter_context(tc.tile_pool(name="ps128", bufs=2, space="PSUM"))
    dram = ctx.enter_context(tc.tile_pool(name="dram", bufs=1, space="DRAM"))

    # ---- constants ----
    w1s = consts.tile([128, n_in_ch, HID2], BF16)
    nc.sync.dma_start(w1s[:], io["w1"].rearrange("(c p) n -> p c n", p=128))
    w2s = consts.tile([128, n_h_ch, HID], BF16)
    nc.sync.dma_start(w2s[:], io["w2"].rearrange("(c p) n -> p c n", p=128))
    bias1 = consts.tile([128, 3, HID2], F32)
    nc.sync.dma_start(bias1[:], io["bias1"])
    bias2 = consts.tile([128, 3, HID], F32)
    nc.sync.dma_start(bias2[:], io["bias2"])
    ident = consts.tile([128, 128], F32)
    nc.sync.dma_start(ident[:], io["ident"])
    iota_b = consts.tile([128, 128], BF16)
    nc.sync.dma_start(iota_b[:], io["iota_b"])
    dinv_t = consts.tile([128, NT], F32)
    nc.sync.dma_start(dinv_t[:], io["dinv_t"])
    dl1 = consts.tile([128, geom["NB1"]], BF16)
    nc.sync.dma_start(dl1[:], io["dl1"])
    dl2 = consts.tile([128, geom["NB2"]], BF16)
    nc.sync.dma_start(dl2[:], io["dl2"])
    eps_t = consts.tile([128, 1], F32)
    nc.vector.memset(eps_t[:], eps)
    c2048 = consts.tile([1, 1], dt.int32)
    nc.sync.dma_start(c2048[:], io["c2048"])
    r2048 = nc.alloc_register(mybir.EngineType.Pool, "gNI")
    nc.gpsimd.reg_load(r2048, c2048[:1, :1])

    # ---- DRAM buffers ----
    tab1c = [dram.tile([CH, HID2], BF16, name=f"tab1c{i}") for i in range(NCHUNK)]
    ag2_in_h = [dram.tile([SHARD // 2, HID], BF16, name=f"ag2in{i}")
                for i in range(2)]
    ag2_out_h = [dram.tile([PADN // 2, HID], BF16, addr_space="Shared",
                           name=f"ag2out{i}") for i in range(2)]

    # ---- stage A: full local table1 = dinv * (X @ W1), bf16; 4 tiles/DMA ----
    for tb in range(0, NT, 14):
        xt_t = work.tile([128, n_in_ch, 14 * 128], BF16, tag="xt")
        nc.sync.dma_start(
            xt_t[:],
            io["xt"][:, tb * 128:(tb + 14) * 128].rearrange("(c p) n -> p c n", p=128))
        h1t = work.tile([128, 14, HID2], BF16, tag="h1t")
        for u in range(14):
            t = tb + u
            ps = ps256.tile([128, HID2], F32, tag="psA")
            for cc in range(n_in_ch):
                nc.tensor.matmul(ps[:], xt_t[:, cc, u * 128:(u + 1) * 128],
                                 w1s[:, cc, :],
                                 start=(cc == 0), stop=(cc == n_in_ch - 1))
            nc.vector.tensor_scalar(h1t[:, u, :], ps[:],
                                    dinv_t[:, t:t + 1], None, AOT.mult)
        tc_, tr = divmod(tb * 128, CH)
        nc.sync.dma_start(
            tab1c[tc_][tr:tr + 14 * 128, :].rearrange("(b p) f -> p b f", p=128),
            h1t[:])

    # ---- generic aggregation layer ----
    def agg_layer(tab_list, feat, B, S, CB, NC, dl_t, io_idx, bias_t, gelu, trow_src, out_cb):
        # emit all gather calls (Tile pipelines via pool WAR deps).
        # Warmup: two windows per chunk in chunk-completion order, so the Q7
        # gathers from early chunks while stage A still builds later chunks.
        msg_tiles = {}
        maxw = int(max(NC))
        WARM = 2
        emit_order = [(cc, w) for cc in range(NCHUNK)
                      for w in range(min(WARM, int(NC[cc])))]
        emit_order += [(cc, w) for w in range(WARM, maxw)
                       for cc in range(NCHUNK) if w < NC[cc]]
        for cc, w in emit_order:
            if True:
                it = idxp.tile([128, NI // 16], dt.int16, tag="idx")
                col0 = int(CB[cc] + w * CALLB) * 8
                nc.sync.dma_start(it[:], io_idx[:, col0:col0 + NI // 16])
                msg = msgp.tile([128, CALLB, feat], BF16, tag="msg")
                nc.gpsimd.dma_gather(
                    msg[:], tab_list[cc][:], it[:],
                    NI, r2048, feat, single_packet=False)
                msg_tiles[(cc, w)] = msg

        for lt in range(TPC):
            bt = int(B[lt].sum())
            assert bt > 0
            ps = ps256.tile([128, feat], F32, tag="psAgg")
            done = 0
            for cc in range(NCHUNK):
                bc = int(B[lt, cc])
                if bc == 0:
                    continue
                sel = selp.tile([128, bc, 128], BF16, tag="sel")
                g0 = int(CB[cc] + S[lt, cc])
                nc.vector.tensor_tensor(
                    sel[:],
                    iota_b[:].rearrange("p (b m) -> p b m", b=1).to_broadcast((128, bc, 128)),
                    dl_t[:, g0:g0 + bc].rearrange("p (b m) -> p b m", m=1).to_broadcast((128, bc, 128)),
                    AOT.is_equal)
                for bi in range(bc):
                    w, j = divmod(g0 + bi, CALLB)
                    w -= int(CB[cc]) // CALLB
                    msg = msg_tiles[(cc, w)]
                    nc.tensor.matmul(ps[:], sel[:, bi, :], msg[:, j, :],
                                     start=(done == 0), stop=(done == bt - 1))
                    done += 1
            # + own row (self loop), scale by dinv[dst], +bias, LN (+gelu)
            trow = trowp.tile([128, feat], BF16, tag="trow")
            if trow_src is not None:
                nc.sync.dma_start(trow[:], trow_src[lt * 128:(lt + 1) * 128, :])
            else:
                hf, lr = divmod(lt, TPC // 2)
                nc.sync.dma_start(trow[:],
                                  ag2_in_h[hf][lr * 128:(lr + 1) * 128, :])
            t_mine = (NT - TPC) + lt  # position of my lt-th tile in per-core order
            xbA = ln.tile([128, feat], F32, tag="xbA")
            nc.vector.tensor_tensor(xbA[:], ps[:], trow[:], AOT.add)
            xb = ln.tile([128, feat], F32, tag="xb")
            r1 = ln.tile([128, 1], F32, tag="r1")
            nc.vector.scalar_tensor_tensor(xb[:], xbA[:], dinv_t[:, t_mine:t_mine + 1],
                                           bias_t[:, 0, :], AOT.mult, AOT.add,
                                           accum_out=r1[:])
            sq = ln.tile([128, feat], F32, tag="sq")
            r2 = ln.tile([128, 1], F32, tag="r2")
            nc.scalar.activation(sq[:], xb[:], AFT.Square, accum_out=r2[:])
            mu = ln.tile([128, 1], F32, tag="mu")
            nc.vector.tensor_scalar(mu[:], r1[:], 1.0 / feat, None, AOT.mult)
            musq = ln.tile([128, 1], F32, tag="musq")
            nc.vector.tensor_tensor(musq[:], mu[:], mu[:], AOT.mult)
            var = ln.tile([128, 1], F32, tag="var")
            nc.vector.tensor_scalar(var[:], r2[:], 1.0 / feat, musq[:],
                                    AOT.mult, AOT.subtract)
            st = ln.tile([128, 1], F32, tag="st")
            nc.scalar.activation(st[:], var[:], AFT.Sqrt, bias=eps_t[:])
            rstd = ln.tile([128, 1], F32, tag="rstd")
            nc.vector.reciprocal(rstd[:], st[:])
            xn = ln.tile([128, feat], F32, tag="xn")
            nc.vector.tensor_scalar(xn[:], xb[:], mu[:], rstd[:],
                                    AOT.subtract, AOT.mult)
            y = ln.tile([128, feat], F32, tag="y")
            nc.vector.tensor_tensor(y[:], xn[:], bias_t[:, 1, :], AOT.mult)
            nc.vector.tensor_tensor(y[:], y[:], bias_t[:, 2, :], AOT.add)
            if gelu:
                h = ln.tile([128, feat], F32, tag="h")
                nc.scalar.activation(h[:], y[:], AFT.Gelu)
                out_cb(lt, h)
            else:
                out_cb(lt, y)

    # ---- L1 -> transform to table2 rows (dinv * h1g @ W2) ----
    def l1_out(lt, h):
        t_mine = (NT - TPC) + lt
        h1T = work.tile([128, n_h_ch, 128], BF16, tag="h1T")
        for cc in range(n_h_ch):
            pst = ps128.tile([128, 128], F32, tag="psT")
            nc.tensor.transpose(pst[:], h[:, cc * 128:(cc + 1) * 128], ident[:])
            nc.vector.tensor_copy(h1T[:, cc, :], pst[:])
        ps2 = ps128.tile([128, HID], F32, tag="psC")
        for cc in range(n_h_ch):
            nc.tensor.matmul(ps2[:], h1T[:, cc, :], w2s[:, cc, :],
                             start=(cc == 0), stop=(cc == n_h_ch - 1))
        h2 = work.tile([128, HID], BF16, tag="h2")
        nc.scalar.activation(h2[:], ps2[:], AFT.Copy, scale=dinv_t[:, t_mine:t_mine + 1])
        hf, lr = divmod(lt, TPC // 2)
        nc.sync.dma_start(ag2_in_h[hf][lr * 128:(lr + 1) * 128, :], h2[:])
        if lt == TPC // 2 - 1:
            # first half of the shard is complete: start its AllGather now
            nc.gpsimd.collective_compute(
                "AllGather", mybir.AluOpType.bypass,
                replica_groups=[list(range(N_CORES))],
                ins=[ag2_in_h[0].opt()], outs=[ag2_out_h[0].opt()])

    agg_layer(tab1c, HID2, geom["B1"], geom["S1"], geom["CB1"], geom["NC1"],
              dl1, io["idx1"], bias1, True, tab1c[3][MYBASE - 3 * CH:, :], l1_out)

    nc.gpsimd.collective_compute(
        "AllGather", AOT.bypass,
        replica_groups=[list(range(N_CORES))],
        ins=[ag2_in_h[1].opt()], outs=[ag2_out_h[1].opt()])

    # ---- L2 aggregation -> final output ----
    def l2_out(lt, y):
        o = work.tile([128, HID], F32, tag="o")
        nc.vector.tensor_copy(o[:], y[:])
        nc.sync.dma_start(io["out"][lt * 128:(lt + 1) * 128, :], o[:])

    tab2_list = [ag2_out_h[cc // 2][(cc % 2) * CH:(cc % 2 + 1) * CH, :]
                 for cc in range(NCHUNK)]
    agg_layer(tab2_list, HID, geom["B2"], geom["S2"], geom["CB2"], geom["NC2"],
              dl2, io["idx2"], bias2, False, None, l2_out)
    ctx.close()


# ============================ top-level kernel ============================

def declare_io(nc, geom):
    io = {
        "xt": nc.dram_tensor("xt", [IN_DIM, PADN], BF16, kind="ExternalInput").ap(),
        "w1": nc.dram_tensor("w1", [IN_DIM, HID2], BF16, kind="ExternalInput").ap(),
        "w2": nc.dram_tensor("w2", [HID2, HID], BF16, kind="ExternalInput").ap(),
        "bias1": nc.dram_tensor("bias1", [128, 3, HID2], F32, kind="ExternalInput").ap(),
        "bias2": nc.dram_tensor("bias2", [128, 3, HID], F32, kind="ExternalInput").ap(),
        "iota_b": nc.dram_tensor("iota_b", [128, 128], BF16, kind="ExternalInput").ap(),
        "ident": nc.dram_tensor("ident", [128, 128], F32, kind="ExternalInput").ap(),
        "dinv_t": nc.dram_tensor("dinv_t", [128, NT], F32, kind="ExternalInput").ap(),
        "idx1": nc.dram_tensor("idx1", [128, geom["NB1"] * 8], dt.int16,
                               kind="ExternalInput").ap(),
        "dl1": nc.dram_tensor("dl1", [128, geom["NB1"]], BF16, kind="ExternalInput").ap(),
        "idx2": nc.dram_tensor("idx2", [128, geom["NB2"] * 8], dt.int16,
                               kind="ExternalInput").ap(),
        "dl2": nc.dram_tensor("dl2", [128, geom["NB2"]], BF16, kind="ExternalInput").ap(),
        "c2048": nc.dram_tensor("c2048", [1, 1], dt.int32, kind="ExternalInput").ap(),
        "out": nc.dram_tensor("out", [SHARD, HID], F32, kind="ExternalOutput").ap(),
    }
    return io


def kernel(x, edge_index, W1, b1, g1, be1, W2, b2, g2, be2,
           trace=False, _return_raw=False):
    bf = dt.np(BF16)
    x = np.asarray(x, np.float32)
    src = np.asarray(edge_index[0], np.int64)
    dst = np.asarray(edge_index[1], np.int64)
    N = x.shape[0]

    deg = (np.bincount(dst, minlength=N) + 1).astype(np.float32)
    dinv = (1.0 / np.sqrt(deg)).astype(np.float32)

    order = np.argsort(-deg, kind="stable")
    node_tile = np.empty(N, np.int32)
    node_slot = np.empty(N, np.int32)
    ar = np.arange(N, dtype=np.int64)
    node_tile[order] = (ar % NT).astype(np.int32)
    node_slot[order] = (ar // NT).astype(np.int32)
    core_of = node_tile % N_CORES
    lt_of = node_tile // N_CORES

    dinv_st = np.ones((TILE, NT), np.float32)
    dinv_st[node_slot, node_tile] = dinv
    halfn = lt_of.astype(np.int64) // (TPC // 2)
    row2 = (halfn * (PADN // 2) + core_of.astype(np.int64) * (SHARD // 2)
            + (lt_of.astype(np.int64) % (TPC // 2)) * TILE + node_slot)

    # --- per-core packing ---
    cores = []
    cnts1, cnts2 = [], []
    for k in range(N_CORES):
        others = np.setdiff1d(np.arange(NT, dtype=np.int64),
                              np.arange(k, NT, N_CORES, dtype=np.int64),
                              assume_unique=True)
        mine = np.arange(k, NT, N_CORES, dtype=np.int64)
        tord = np.concatenate([others, mine])
        tpos = np.empty(NT, np.int64)
        tpos[tord] = np.arange(NT, dtype=np.int64)
        row1 = tpos[node_tile] * TILE + node_slot

        m = core_of[dst] == k
        elt = lt_of[dst[m]].astype(np.int64)
        eslot = node_slot[dst[m]].astype(np.float32)
        esrc = src[m]

        def sort_pack(srcrow):
            c = srcrow // CH
            i16 = (srcrow - c * CH).astype(np.int16)
            key = elt * NCHUNK + c
            o = np.argsort(key, kind="stable")
            cnts = np.bincount(key, minlength=TPC * NCHUNK).reshape(TPC, NCHUNK)
            return i16[o], eslot[o], cnts

        i16a, sla, ca = sort_pack(row1[esrc])
        i16b, slb, cb = sort_pack(row2[esrc])
        cnts1.append(ca)
        cnts2.append(cb)

        xs = np.zeros((PADN, IN_DIM), np.float32)
        xs[row1] = x
        cores.append(dict(
            xt=np.ascontiguousarray(xs.T).astype(bf),
            dinv_t=np.ascontiguousarray(dinv_st[:, tord]),
            e1=(i16a, sla, ca), e2=(i16b, slb, cb),
            nodes=np.nonzero(core_of == k)[0]))

    B1, S1, CB1, NC1, NB1 = finalize_geometry(cnts1)
    B2, S2, CB2, NC2, NB2 = finalize_geometry(cnts2)
    geom = dict(B1=B1, S1=S1, CB1=CB1, NC1=NC1, NB1=NB1,
                B2=B2, S2=S2, CB2=CB2, NC2=NC2, NB2=NB2)

    iota_np = np.tile(np.arange(128, dtype=np.float32)[None, :], (128, 1)).astype(bf)
    ident_np = np.eye(128, dtype=np.float32)
    bias1_np = np.broadcast_to(
        np.stack([np.asarray(b1, np.float32), np.asarray(g1, np.float32),
                  np.asarray(be1, np.float32)])[None], (128, 3, HID2)).copy()
    bias2_np = np.broadcast_to(
        np.stack([np.asarray(b2, np.float32), np.asarray(g2, np.float32),
                  np.asarray(be2, np.float32)])[None], (128, 3, HID)).copy()

    in_maps = []
    for k in range(N_CORES):
        pc = cores[k]
        idx1, dl1 = build_core_arrays(pc["e1"], B1, S1, CB1, NB1)
        idx2, dl2 = build_core_arrays(pc["e2"], B2, S2, CB2, NB2)
        in_maps.append({
            "xt": pc["xt"], "w1": np.asarray(W1, np.float32).astype(bf),
            "w2": np.asarray(W2, np.float32).astype(bf),
            "bias1": bias1_np, "bias2": bias2_np,
            "iota_b": iota_np, "ident": ident_np,
            "dinv_t": pc["dinv_t"],
            "idx1": idx1, "dl1": dl1, "idx2": idx2, "dl2": dl2,
            "c2048": np.array([[NI]], np.int32),
        })

    nc = bacc.Bacc("TRN2", debug=False, num_devices=N_CORES)
    io = declare_io(nc, geom)
    with tile.TileContext(nc) as tc:
        build_program(tc, io, geom)
    nc.compile()

    res = run_bass_kernel_spmd(nc, in_maps, core_ids=list(range(N_CORES)),
                               trace=trace)
    out = np.empty((N, HID), np.float32)
    for k in range(N_CORES):
        pc = cores[k]
        ok = np.asarray(res.results[k]["out"])
        pos = lt_of[pc["nodes"]] * TILE + node_slot[pc["nodes"]]
        out[pc["nodes"]] = ok[pos]
    if _return_raw:
        return out, res
    return out


def build_core_arrays(epack, B, S, CB, NB):
    bf = dt.np(BF16)
    i16, slot, cnts = epack
    idx_a = np.zeros((16, NB * 8), np.int16)
    dl_a = np.full((TILE, NB), PADSLOT, np.float32)
    starts = np.zeros(TPC * NCHUNK + 1, np.int64)
    np.cumsum(cnts.reshape(-1), out=starts[1:])
    for lt in range(TPC):
        for cc in range(NCHUNK):
            m = int(cnts[lt, cc])
            if m == 0:
                continue
            s0 = int(starts[lt * NCHUNK + cc])
            p = (int(CB[cc] + S[lt, cc])) * TILE + np.arange(m)
            idx_a[p % 16, p // 16] = i16[s0:s0 + m]
            dl_a[p % TILE, p // TILE] = slot[s0:s0 + m]
    return np.tile(idx_a, (8, 1)), dl_a.astype(bf)


def finalize_geometry(cnts_list):
    allc = np.stack(cnts_list)  # [8, TPC, NCHUNK]
    B = (-(-allc.max(axis=0) // TILE)).astype(np.int64)
    S = np.zeros((TPC, NCHUNK), np.int64)
    CB = np.zeros(NCHUNK + 1, np.int64)
    NC = np.zeros(NCHUNK, np.int64)
    for cc in range(NCHUNK):
        S[:, cc] = np.cumsum(B[:, cc]) - B[:, cc]
        nb = int(B[:, cc].sum())
        NC[cc] = -(-nb // CALLB)
        CB[cc + 1] = CB[cc] + NC[cc] * CALLB
    return B, S, CB, NC, int(CB[NCHUNK])


# revision 17
# speedup vs baseline: 1.0212x; 1.0212x over previous
"""Trainium2 Bass kernel for a 2-layer GCN encoder (GCNConv -> LN -> GELU -> GCNConv -> LN).

Strategy (8 NeuronCores, SPMD), v2:
  - Nodes assigned to 784 global tiles of 128 (degree-balanced); core k owns
    tiles t with t%8==k (98 tiles = 12544 dst rows per core).
  - Layer 1: every core computes the FULL transformed table H1*dinv locally
    (X@W1 on all 784 tiles; no collective). Per-core table row order is a
    per-core permutation with the core's own tiles LAST so "my rows" sit at a
    core-independent offset.
  - Aggregation: normalization folded into the table (rows pre-scaled by
    dinv[src]) and the output (post-scaled by dinv[dst]); selector matrices
    are pure one-hot (single is_equal). Self-loops handled by adding the
    node's own table row (sequential read), not as gather edges.
  - Gathers: bf16 rows via dma_gather in fixed 3072-idx multi-packet calls,
    fully padded (pad idx=0, pad slot=200 -> zero selector column), so no
    per-call count registers and no NaN-guard memsets.
  - Layer 2: transform locally (h1g @ W2, scaled by dinv), AllGather the
    bf16 table (core-major rows), aggregate the same way.
"""

from contextlib import ExitStack

import numpy as np

import concourse.bass as bass
import concourse.bacc as bacc
import concourse.mybir as mybir
import concourse.tile as tile
from concourse.bass_utils import run_bass_kernel_spmd

dt = mybir.dt
F32 = dt.float32
BF16 = dt.bfloat16

# -------- problem geometry (hardcoded for the graded problem) --------
N_FULL = 100000
IN_DIM = 256
HID2 = 256
HID = 128
N_CORES = 8
TILE = 128
NT = 784           # global tiles
TPC = 98           # tiles per core
SHARD = TPC * TILE # 12544
PADN = NT * TILE   # 100352
NCHUNK = 4
CH = PADN // NCHUNK  # 25088 (int16-safe)
CALLB = 24         # blocks per gather call (3072 idxs; multi-packet)
NI = CALLB * TILE  # 3072
MYBASE = (NT - TPC) * TILE  # 87808: per-core table rows of own tiles
PADSLOT = 200.0


# ============================ bass program builder ============================

def build_program(tc, io, geom):
    nc = tc.nc
    AOT = mybir.AluOpType
    AFT = mybir.ActivationFunctionType
    eps = 1e-5
    n_in_ch = IN_DIM // 128
    n_h_ch = HID2 // 128

    ctx = ExitStack()
    consts = ctx.enter_context(tc.tile_pool(name="consts", bufs=1))
    work = ctx.enter_context(tc.tile_pool(name="work", bufs=2))
    ln = ctx.enter_context(tc.tile_pool(name="ln", bufs=3))
    msgp = ctx.enter_context(tc.tile_pool(name="msgp", bufs=8))
    idxp = ctx.enter_context(tc.tile_pool(name="idxp", bufs=4))
    selp = ctx.enter_context(tc.tile_pool(name="selp", bufs=3))
    trowp = ctx.enter_context(tc.tile_pool(name="trowp", bufs=2))
    ps256 = ctx.enter_context(tc.tile_pool(name="ps256", bufs=2, space="PSUM"))
    ps128 = ctx.enter_context(tc.tile_pool(name="ps128", bufs=2, space="PSUM"))
    dram = ctx.enter_context(tc.tile_pool(name="dram", bufs=1, space="DRAM"))

    # ---- constants ----
    w1s = consts.tile([128, n_in_ch, HID2], BF16)
    nc.sync.dma_start(w1s[:], io["w1"].rearrange("(c p) n -> p c n", p=128))
    w2s = consts.tile([128, n_h_ch, HID], BF16)
    nc.sync.dma_start(w2s[:], io["w2"].rearrange("(c p) n -> p c n", p=128))
    bias1 = consts.tile([128, 3, HID2], F32)
    nc.sync.dma_start(bias1[:], io["bias1"])
    bias2 = consts.tile([128, 3, HID], F32)
    nc.sync.dma_start(bias2[:], io["bias2"])
    ident = consts.tile([128, 128], F32)
    nc.sync.dma_start(ident[:], io["ident"])
    iota_b = consts.tile([128, 128], BF16)
    nc.sync.dma_start(iota_b[:], io["iota_b"])
    dinv_t = consts.tile([128, NT], F32)
    nc.sync.dma_start(dinv_t[:], io["dinv_t"])
    dl1 = consts.tile([128, geom["NB1"]], BF16)
    nc.sync.dma_start(dl1[:], io["dl1"])
    dl2 = consts.tile([128, geom["NB2"]], BF16)
    nc.sync.dma_start(dl2[:], io["dl2"])
    eps_t = consts.tile([128, 1], F32)
    nc.vector.memset(eps_t[:], eps)
    c2048 = consts.tile([1, 1], dt.int32)
    nc.sync.dma_start(c2048[:], io["c2048"])
    r2048 = nc.alloc_register(mybir.EngineType.Pool, "gNI")
    nc.gpsimd.reg_load(r2048, c2048[:1, :1])

    # ---- DRAM buffers ----
    tab1c = [dram.tile([CH, HID2], BF16, name=f"tab1c{i}") for i in range(NCHUNK)]
    ag2_in_h = [dram.tile([SHARD // 2, HID], BF16, name=f"ag2in{i}")
                for i in range(2)]
    ag2_out_h = [dram.tile([PADN // 2, HID], BF16, addr_space="Shared",
                           name=f"ag2out{i}") for i in range(2)]

    # ---- stage A: full local table1 = dinv * (X @ W1), bf16; 4 tiles/DMA ----
    for tb in range(0, NT, 14):
        xt_t = work.tile([128, n_in_ch, 14 * 128], BF16, tag="xt")
        nc.sync.dma_start(
            xt_t[:],
            io["xt"][:, tb * 128:(tb + 14) * 128].rearrange("(c p) n -> p c n", p=128))
        h1t = work.tile([128, 14, HID2], BF16, tag="h1t")
        for u in range(14):
            t = tb + u
            ps = ps256.tile([128, HID2], F32, tag="psA")
            for cc in range(n_in_ch):
                nc.tensor.matmul(ps[:], xt_t[:, cc, u * 128:(u + 1) * 128],
                                 w1s[:, cc, :],
                                 start=(cc == 0), stop=(cc == n_in_ch - 1))
            nc.vector.tensor_scalar(h1t[:, u, :], ps[:],
                                    dinv_t[:, t:t + 1], None, AOT.mult)
        tc_, tr = divmod(tb * 128, CH)
        nc.sync.dma_start(
            tab1c[tc_][tr:tr + 14 * 128, :].rearrange("(b p) f -> p b f", p=128),
            h1t[:])

    # ---- generic aggregation layer ----
    def agg_layer(tab_list, feat, B, S, CB, NC, dl_t, io_idx, bias_t, gelu, trow_src, out_cb):
        # emit all gather calls (Tile pipelines via pool WAR deps).
        # Warmup: two windows per chunk in chunk-completion order, so the Q7
        # gathers from early chunks while stage A still builds later chunks.
        msg_tiles = {}
        maxw = int(max(NC))
        WARM = 2
        emit_order = [(cc, w) for cc in range(NCHUNK)
                      for w in range(min(WARM, int(NC[cc])))]
        emit_order += [(cc, w) for w in range(WARM, maxw)
                       for cc in range(NCHUNK) if w < NC[cc]]
        for cc, w in emit_order:
            if True:
                it = idxp.tile([128, NI // 16], dt.int16, tag="idx")
                col0 = int(CB[cc] + w * CALLB) * 8
                nc.sync.dma_start(it[:], io_idx[:, col0:col0 + NI // 16])
                msg = msgp.tile([128, CALLB, feat], BF16, tag="msg")
                nc.gpsimd.dma_gather(
                    msg[:], tab_list[cc][:], it[:],
                    NI, r2048, feat, single_packet=False)
                msg_tiles[(cc, w)] = msg

        for lt in range(TPC):
            bt = int(B[lt].sum())
            assert bt > 0
            ps = ps256.tile([128, feat], F32, tag="psAgg")
            done = 0
            for cc in range(NCHUNK):
                bc = int(B[lt, cc])
                if bc == 0:
                    continue
                sel = selp.tile([128, bc, 128], BF16, tag="sel")
                g0 = int(CB[cc] + S[lt, cc])
                nc.vector.tensor_tensor(
                    sel[:],
                    iota_b[:].rearrange("p (b m) -> p b m", b=1).to_broadcast((128, bc, 128)),
                    dl_t[:, g0:g0 + bc].rearrange("p (b m) -> p b m", m=1).to_broadcast((128, bc, 128)),
                    AOT.is_equal)
                for bi in range(bc):
                    w, j = divmod(g0 + bi, CALLB)
                    w -= int(CB[cc]) // CALLB
                    msg = msg_tiles[(cc, w)]
                    nc.tensor.matmul(ps[:], sel[:, bi, :], msg[:, j, :],
                                     start=(done == 0), stop=(done == bt - 1))
                    done += 1
            # + own row (self loop), scale by dinv[dst], +bias, LN (+gelu)
            trow = trowp.tile([128, feat], BF16, tag="trow")
            if trow_src is not None:
                nc.sync.dma_start(trow[:], trow_src[lt * 128:(lt + 1) * 128, :])
            else:
                hf, lr = divmod(lt, TPC // 2)
                nc.sync.dma_start(trow[:],
                                  ag2_in_h[hf][lr * 128:(lr + 1) * 128, :])
            t_mine = (NT - TPC) + lt  # position of my lt-th tile in per-core order
            xbA = ln.tile([128, feat], F32, tag="xbA")
            nc.vector.tensor_tensor(xbA[:], ps[:], trow[:], AOT.add)
            xb = ln.tile([128, feat], F32, tag="xb")
            r1 = ln.tile([128, 1], F32, tag="r1")
            nc.vector.scalar_tensor_tensor(xb[:], xbA[:], dinv_t[:, t_mine:t_mine + 1],
                                           bias_t[:, 0, :], AOT.mult, AOT.add,
                                           accum_out=r1[:])
            sq = ln.tile([128, feat], F32, tag="sq")
            r2 = ln.tile([128, 1], F32, tag="r2")
            nc.scalar.activation(sq[:], xb[:], AFT.Square, accum_out=r2[:])
            mu = ln.tile([128, 1], F32, tag="mu")
            nc.vector.tensor_scalar(mu[:], r1[:], 1.0 / feat, None, AOT.mult)
            musq = ln.tile([128, 1], F32, tag="musq")
            nc.vector.tensor_tensor(musq[:], mu[:], mu[:], AOT.mult)
            var = ln.tile([128, 1], F32, tag="var")
            nc.vector.tensor_scalar(var[:], r2[:], 1.0 / feat, musq[:],
                                    AOT.mult, AOT.subtract)
            st = ln.tile([128, 1], F32, tag="st")
            nc.scalar.activation(st[:], var[:], AFT.Sqrt, bias=eps_t[:])
            rstd = ln.tile([128, 1], F32, tag="rstd")
            nc.vector.reciprocal(rstd[:], st[:])
            xn = ln.tile([128, feat], F32, tag="xn")
            nc.vector.tensor_scalar(xn[:], xb[:], mu[:], rstd[:],
                                    AOT.subtract, AOT.mult)
            y = ln.tile([128, feat], F32, tag="y")
            nc.vector.tensor_tensor(y[:], xn[:], bias_t[:, 1, :], AOT.mult)
            nc.vector.tensor_tensor(y[:], y[:], bias_t[:, 2, :], AOT.add)
            if gelu:
                h = ln.tile([128, feat], F32, tag="h")
                nc.scalar.activation(h[:], y[:], AFT.Gelu)
                out_cb(lt, h)
            else:
                out_cb(lt, y)

    # ---- L1 -> transform to table2 rows (dinv * h1g @ W2) ----
    def l1_out(lt, h):
        t_mine = (NT - TPC) + lt
        h1T = work.tile([128, n_h_ch, 128], BF16, tag="h1T")
        for cc in range(n_h_ch):
            pst = ps128.tile([128, 128], F32, tag="psT")
            nc.tensor.transpose(pst[:], h[:, cc * 128:(cc + 1) * 128], ident[:])
            nc.vector.tensor_copy(h1T[:, cc, :], pst[:])
        ps2 = ps128.tile([128, HID], F32, tag="psC")
        for cc in range(n_h_ch):
            nc.tensor.matmul(ps2[:], h1T[:, cc, :], w2s[:, cc, :],
                             start=(cc == 0), stop=(cc == n_h_ch - 1))
        h2 = work.tile([128, HID], BF16, tag="h2")
        nc.scalar.activation(h2[:], ps2[:], AFT.Copy, scale=dinv_t[:, t_mine:t_mine + 1])
        hf, lr = divmod(lt, TPC // 2)
        nc.sync.dma_start(ag2_in_h[hf][lr * 128:(lr + 1) * 128, :], h2[:])

    agg_layer(tab1c, HID2, geom["B1"], geom["S1"], geom["CB1"], geom["NC1"],
              dl1, io["idx1"], bias1, True, tab1c[3][MYBASE - 3 * CH:, :], l1_out)

    for hf in range(2):
        nc.gpsimd.collective_compute(
            "AllGather", AOT.bypass,
            replica_groups=[list(range(N_CORES))],
            ins=[ag2_in_h[hf].opt()], outs=[ag2_out_h[hf].opt()])

    # ---- L2 aggregation -> final output ----
    def l2_out(lt, y):
        o = work.tile([128, HID], F32, tag="o")
        nc.vector.tensor_copy(o[:], y[:])
        nc.sync.dma_start(io["out"][lt * 128:(lt + 1) * 128, :], o[:])

    tab2_list = [ag2_out_h[cc // 2][(cc % 2) * CH:(cc % 2 + 1) * CH, :]
                 for cc in range(NCHUNK)]
    agg_layer(tab2_list, HID, geom["B2"], geom["S2"], geom["CB2"], geom["NC2"],
              dl2, io["idx2"], bias2, False, None, l2_out)
    ctx.close()


# ============================ top-level kernel ============================

def declare_io(nc, geom):
    io = {
        "xt": nc.dram_tensor("xt", [IN_DIM, PADN], BF16, kind="ExternalInput").ap(),
        "w1": nc.dram_tensor("w1", [IN_DIM, HID2], BF16, kind="ExternalInput").ap(),
        "w2": nc.dram_tensor("w2", [HID2, HID], BF16, kind="ExternalInput").ap(),
        "bias1": nc.dram_tensor("bias1", [128, 3, HID2], F32, kind="ExternalInput").ap(),
        "bias2": nc.dram_tensor("bias2", [128, 3, HID], F32, kind="ExternalInput").ap(),
        "iota_b": nc.dram_tensor("iota_b", [128, 128], BF16, kind="ExternalInput").ap(),
        "ident": nc.dram_tensor("ident", [128, 128], F32, kind="ExternalInput").ap(),
        "dinv_t": nc.dram_tensor("dinv_t", [128, NT], F32, kind="ExternalInput").ap(),
        "idx1": nc.dram_tensor("idx1", [128, geom["NB1"] * 8], dt.int16,
                               kind="ExternalInput").ap(),
        "dl1": nc.dram_tensor("dl1", [128, geom["NB1"]], BF16, kind="ExternalInput").ap(),
        "idx2": nc.dram_tensor("idx2", [128, geom["NB2"] * 8], dt.int16,
                               kind="ExternalInput").ap(),
        "dl2": nc.dram_tensor("dl2", [128, geom["NB2"]], BF16, kind="ExternalInput").ap(),
        "c2048": nc.dram_tensor("c2048", [1, 1], dt.int32, kind="ExternalInput").ap(),
        "out": nc.dram_tensor("out", [SHARD, HID], F32, kind="ExternalOutput").ap(),
    }
    return io


def kernel(x, edge_index, W1, b1, g1, be1, W2, b2, g2, be2,
           trace=False, _return_raw=False):
    bf = dt.np(BF16)
    x = np.asarray(x, np.float32)
    src = np.asarray(edge_index[0], np.int64)
    dst = np.asarray(edge_index[1], np.int64)
    N = x.shape[0]

    deg = (np.bincount(dst, minlength=N) + 1).astype(np.float32)
    dinv = (1.0 / np.sqrt(deg)).astype(np.float32)

    order = np.argsort(-deg, kind="stable")
    node_tile = np.empty(N, np.int32)
    node_slot = np.empty(N, np.int32)
    ar = np.arange(N, dtype=np.int64)
    node_tile[order] = (ar % NT).astype(np.int32)
    node_slot[order] = (ar // NT).astype(np.int32)
    core_of = node_tile % N_CORES
    lt_of = node_tile // N_CORES

    dinv_st = np.ones((TILE, NT), np.float32)
    dinv_st[node_slot, node_tile] = dinv
    halfn = lt_of.astype(np.int64) // (TPC // 2)
    row2 = (halfn * (PADN // 2) + core_of.astype(np.int64) * (SHARD // 2)
            + (lt_of.astype(np.int64) % (TPC // 2)) * TILE + node_slot)

    # --- per-core packing ---
    cores = []
    cnts1, cnts2 = [], []
    for k in range(N_CORES):
        others = np.setdiff1d(np.arange(NT, dtype=np.int64),
                              np.arange(k, NT, N_CORES, dtype=np.int64),
                              assume_unique=True)
        mine = np.arange(k, NT, N_CORES, dtype=np.int64)
        tord = np.concatenate([others, mine])
        tpos = np.empty(NT, np.int64)
        tpos[tord] = np.arange(NT, dtype=np.int64)
        row1 = tpos[node_tile] * TILE + node_slot

        m = core_of[dst] == k
        elt = lt_of[dst[m]].astype(np.int64)
        eslot = node_slot[dst[m]].astype(np.float32)
        esrc = src[m]

        def sort_pack(srcrow):
            c = srcrow // CH
            i16 = (srcrow - c * CH).astype(np.int16)
            key = elt * NCHUNK + c
            o = np.argsort(key, kind="stable")
            cnts = np.bincount(key, minlength=TPC * NCHUNK).reshape(TPC, NCHUNK)
            return i16[o], eslot[o], cnts

        i16a, sla, ca = sort_pack(row1[esrc])
        i16b, slb, cb = sort_pack(row2[esrc])
        cnts1.append(ca)
        cnts2.append(cb)

        xs = np.zeros((PADN, IN_DIM), np.float32)
        xs[row1] = x
        cores.append(dict(
            xt=np.ascontiguousarray(xs.T).astype(bf),
            dinv_t=np.ascontiguousarray(dinv_st[:, tord]),
            e1=(i16a, sla, ca), e2=(i16b, slb, cb),
            nodes=np.nonzero(core_of == k)[0]))

    B1, S1, CB1, NC1, NB1 = finalize_geometry(cnts1)
    B2, S2, CB2, NC2, NB2 = finalize_geometry(cnts2)
    geom = dict(B1=B1, S1=S1, CB1=CB1, NC1=NC1, NB1=NB1,
                B2=B2, S2=S2, CB2=CB2, NC2=NC2, NB2=NB2)

    iota_np = np.tile(np.arange(128, dtype=np.float32)[None, :], (128, 1)).astype(bf)
    ident_np = np.eye(128, dtype=np.float32)
    bias1_np = np.broadcast_to(
        np.stack([np.asarray(b1, np.float32), np.asarray(g1, np.float32),
                  np.asarray(be1, np.float32)])[None], (128, 3, HID2)).copy()
    bias2_np = np.broadcast_to(
        np.stack([np.asarray(b2, np.float32), np.asarray(g2, np.float32),
                  np.asarray(be2, np.float32)])[None], (128, 3, HID)).copy()

    in_maps = []
    for k in range(N_CORES):
        pc = cores[k]
        idx1, dl1 = build_core_arrays(pc["e1"], B1, S1, CB1, NB1)
        idx2, dl2 = build_core_arrays(pc["e2"], B2, S2, CB2, NB2)
        in_maps.append({
            "xt": pc["xt"], "w1": np.asarray(W1, np.float32).astype(bf),
            "w2": np.asarray(W2, np.float32).astype(bf),
            "bias1": bias1_np, "bias2": bias2_np,
            "iota_b": iota_np, "ident": ident_np,
            "dinv_t": pc["dinv_t"],
            "idx1": idx1, "dl1": dl1, "idx2": idx2, "dl2": dl2,
            "c2048": np.array([[NI]], np.int32),
        })

    nc = bacc.Bacc("TRN2", debug=False, num_devices=N_CORES)
    io = declare_io(nc, geom)
    with tile.TileContext(nc) as tc:
        build_program(tc, io, geom)
    nc.compile()

    res = run_bass_kernel_spmd(nc, in_maps, core_ids=list(range(N_CORES)),
                               trace=trace)
    out = np.empty((N, HID), np.float32)
    for k in range(N_CORES):
        pc = cores[k]
        ok = np.asarray(res.results[k]["out"])
        pos = lt_of[pc["nodes"]] * TILE + node_slot[pc["nodes"]]
        out[pc["nodes"]] = ok[pos]
    if _return_raw:
        return out, res
    return out


def build_core_arrays(epack, B, S, CB, NB):
    bf = dt.np(BF16)
    i16, slot, cnts = epack
    idx_a = np.zeros((16, NB * 8), np.int16)
    dl_a = np.full((TILE, NB), PADSLOT, np.float32)
    starts = np.zeros(TPC * NCHUNK + 1, np.int64)
    np.cumsum(cnts.reshape(-1), out=starts[1:])
    for lt in range(TPC):
        for cc in range(NCHUNK):
            m = int(cnts[lt, cc])
            if m == 0:
                continue
            s0 = int(starts[lt * NCHUNK + cc])
            p = (int(CB[cc] + S[lt, cc])) * TILE + np.arange(m)
            idx_a[p % 16, p // 16] = i16[s0:s0 + m]
            dl_a[p % TILE, p // TILE] = slot[s0:s0 + m]
    return np.tile(idx_a, (8, 1)), dl_a.astype(bf)


def finalize_geometry(cnts_list):
    allc = np.stack(cnts_list)  # [8, TPC, NCHUNK]
    B = (-(-allc.max(axis=0) // TILE)).astype(np.int64)
    S = np.zeros((TPC, NCHUNK), np.int64)
    CB = np.zeros(NCHUNK + 1, np.int64)
    NC = np.zeros(NCHUNK, np.int64)
    for cc in range(NCHUNK):
        S[:, cc] = np.cumsum(B[:, cc]) - B[:, cc]
        nb = int(B[:, cc].sum())
        NC[cc] = -(-nb // CALLB)
        CB[cc + 1] = CB[cc] + NC[cc] * CALLB
    return B, S, CB, NC, int(CB[NCHUNK])
